# revision 2
# baseline (speedup 1.0000x reference)
"""DistanceCentroidLoss on 8 Trainium2 NeuronCores (Bass/Tile), v2.

Data-parallel over N: each core processes NS=32768 rows (D=128, K=64).

Per 128-row tile, ONE fp8 DoubleRow matmul computes the full (quantized)
d2[n,j] = x2[n] + c2[j] - 2 x.c_j into PSUM:
  - x and cT are packed [64, 2, *] fp8e4m3; DoubleRow contracts pairwise over
    (partition p, ktile q), so any host-side (p,q)->d mapping works as long as
    both operands use the same one (here d = q*64 + p).
  - two extra contraction partitions (64, 65) carry the x2[n]/c2[j] bias terms
    in a two-digit fp8 decomposition v = 16*round(v/16) + r: the coarse digit
    16*a is exactly representable in e4m3, giving ~bf16 accuracy:
      lhsT[64] = (16*x2a[m], 1)   rhs[64] = (1, 16*c2a[j])
      lhsT[65] = (1, x2b[m])      rhs[65] = (c2b[j], 1)
ACT sqrts PSUM -> dist (f16, in dd). DVE builds oh = onehot(labels) and
moh = dist*oh (2x-mode f16). PE accumulates, over all 256 tiles,
  G[j, s*64+k] = sum_n dist[n,j] * ohm[n,s,k]   (s=0: oh, s=1: moh)
so G1=G[:, :64] gives O_k (diag) and W_k (colsum-diag), and G2=G[:, 64:]
diag gives A_k = sum_{n in k} d2_own (since moh*dist = d2 at the label col).
V_k comes from a closed form on the host: row-sums of d2 are linear in x2,
c2 and x.(sum_j c_j), evaluated with the device-effective fp8 values so host
and device quantization match exactly. Final scalar assembled on host.

Toolchain quirk: this walrus build rejects any instruction with more than one
semaphore wait. Single SWDGE completion lane, persistent arenas, PE nops that
pre-observe ACT/DVE sems before each block (so matmuls carry at most the
chunk-DMA wait), emission ordered to need <=1 new wait per instruction, plus
a post-pass stripping semantically-redundant waits Tile emits.
"""
import sys

sys.path.insert(0, '/opt/trn_rl_repo')

import numpy as np
import ml_dtypes

import concourse.bass as bass
import concourse.mybir as mybir
import concourse.tile_sem_assignment as _tsa
from concourse.tile import TileContext, add_dep_helper
from concourse.bass_utils import run_bass_kernel_spmd

f32 = mybir.dt.float32
bf16 = mybir.dt.bfloat16
f16 = mybir.dt.float16
f8 = mybir.dt.float8e4
i32 = mybir.dt.int32
DR = mybir.MatmulPerfMode.DoubleRow
E4M3 = ml_dtypes.float8_e4m3

N, D, K = 262144, 128, 64
NCORES = 8
NS = N // NCORES        # rows per core = 32768
T = NS // 128           # 256 tiles of 128 rows
TPB = 8                 # tiles per block
NB = T // TPB           # 32 blocks
TPC = 32                # tiles per DMA chunk
NCH = T // TPC          # 8 chunks
BPC = TPC // TPB        # blocks per chunk = 4
MARGIN = 5.0
CW = 416                # const pack: f32 words per partition

_cache = {}

_OWN_PREFIX = {
    "Activation": ("Activation_",),
    "DVE": ("DVE_",),
    "PE": ("PE_",),
    "Pool": ("Pool_",),
    "SP": ("SP_sequencer",),
}


def _is_own(eng, name):
    for p in _OWN_PREFIX.get(eng, ()):
        if name.startswith(p) and not name.startswith("barrier"):
            return True
    return False


def _strip_redundant_waits(nc):
    """Drop tautological sem waits (see module docstring)."""
    own, seen = {}, {}
    for blk in nc.m.functions[0].blocks:
        for inst in blk.instructions:
            eng = str(inst.engine).split(".")[-1]
            si = inst.sync_info
            oc = own.setdefault(eng, {})
            ob = seen.setdefault(eng, {})
            if si is not None and si.on_wait:
                keep = []
                for w in si.on_wait:
                    if w.sync_type != "semaphore" or w.wait_mode != "sem-ge-imm" \
                            or w.wait_reg is not None or w.wait_value is None:
                        keep.append(w)
                        continue
                    nm, v = w.ant_name, w.wait_value
                    if "barrier" in nm:
                        # barrier EVSEMs are sem-sub'd (non-monotonic): never elide
                        keep.append(w)
                        continue
                    if ob.get(nm, -1) >= v or (_is_own(eng, nm) and oc.get(nm, 0) >= v):
                        continue
                    keep.append(w)
                    ob[nm] = max(ob.get(nm, -1), v)
                if len(keep) != len(si.on_wait):
                    si.on_wait = keep
                    inst.sync_info = si
            if si is not None and si.on_update:
                dma = "DMA" in type(inst).__name__ or "DmaTranspose" in type(inst).__name__
                for u in si.on_update:
                    if u.update_mode != "sem-inc" or u.update_value is None:
                        continue
                    if not dma and _is_own(eng, u.ant_name):
                        oc[u.ant_name] = oc.get(u.ant_name, 0) + u.update_value
    return nc


def _build():
    # single completion-sem lane for SWDGE: all DMAs share DMASW0, so
    # consumers never carry waits on more than one DMA proc.
    _tsa.NUM_SWDGE_GLOBAL_SEMS = 1
    _tsa.NUM_HWDGE_SEMS = 1

    nc = bass.Bass()
    x8_in = nc.dram_tensor("x8", [66, T * 2 * 128], f8, kind="ExternalInput")
    consts_in = nc.dram_tensor("consts", [128, CW], f32, kind="ExternalInput")
    out_G = nc.dram_tensor("out_G", [K, 2 * K], f32, kind="ExternalOutput")

    with TileContext(nc) as tc:
        with tc.tile_pool(name="single", bufs=1) as sb, \
             tc.tile_pool(name="ohp", bufs=3) as ohp, \
             tc.tile_pool(name="pxc", bufs=2, space="PSUM") as pxc, \
             tc.tile_pool(name="pacc", bufs=1, space="PSUM") as pacc:

            # ---- one packed constant DMA on the SWDGE lane ----
            consts_sb = sb.tile([128, CW], f32)
            cdma = nc.gpsimd.dma_start(out=consts_sb, in_=consts_in[:])
            iota_tbk = consts_sb[:, 0:256].bitcast(f16).rearrange(
                "p (t k) -> p t k", k=K)                       # [128,8,64] f16
            labf_sb = consts_sb[:, 256:384].bitcast(f16)       # [128,256] f16
            ctf8 = consts_sb[:, 384:416].bitcast(f8).rearrange(
                "p (q j) -> p q j", j=K)                       # [128,2,64] fp8

            # ---- persistent per-chunk x arenas + dist store ----
            x8s = [sb.tile([66, TPC, 2, 128], f8, name=f"x8{c}") for c in range(NCH)]
            dd = sb.tile([128, T, K], f16)            # 4 MB: dist only

            G_ps = pacc.tile([K, 2 * K], f32)

            casts = []
            sqrts = []
            blocks = []          # (b, ohm tile, moh inst)
            dve_tail = {"i": None}
            pe_tail = {"i": None}

            def dve_pin(i):
                # Tile's lowering reorders ready instructions within an engine
                # queue; the wait-elision scheme depends on DVE executing in
                # emission order, so chain every DVE op with nosync pins.
                if dve_tail["i"] is not None:
                    add_dep_helper(i.ins, dve_tail["i"].ins, sync=False,
                                   reason="dve order pin")
                dve_tail["i"] = i

            def pe_pin(i):
                if pe_tail["i"] is not None:
                    add_dep_helper(i.ins, pe_tail["i"].ins, sync=False,
                                   reason="pe order pin")
                pe_tail["i"] = i

            def emit_g(entry, is_last):
                # G matmuls for block b, emitted two blocks late so PE never
                # stalls on the sqrt->moh chain. npB pre-observes DVE's moh.
                b, ohm_b, i_moh = entry
                npB = nc.tensor.nop()
                add_dep_helper(npB.ins, i_moh.ins, sync=True,
                               reason="pe observes moh")
                pe_pin(npB)
                for tt in range(TPB):
                    t = b * TPB + tt
                    i_g = nc.tensor.matmul(
                        out=G_ps[:], lhsT=dd[:, t, :], rhs=ohm_b[:, tt, :, :],
                        start=(t == 0),
                        stop=(is_last and tt == TPB - 1),
                        skip_group_check=True)
                    pe_pin(i_g)
                return i_g

            for c in range(NCH):
                cast = nc.gpsimd.dma_start(
                    out=x8s[c],
                    in_=x8_in[:, c * TPC * 256:(c + 1) * TPC * 256].rearrange(
                        "d (t q m) -> d t q m", q=2, m=128))
                casts.append(cast)

                for bb in range(BPC):
                    b = c * BPC + bb
                    if b >= 2:
                        # PE pre-observes sqrt(b-2): covers the psum-bank WAR
                        # so the first xc matmul carries only the DMA wait.
                        npA = nc.tensor.nop()
                        add_dep_helper(npA.ins, sqrts[b - 2].ins, sync=True,
                                       reason="pe observes sqrt WAR")
                        pe_pin(npA)

                    psum = pxc.tile([128, TPB * K], f32, tag="pxc")
                    for tt in range(TPB):
                        i_mm = nc.tensor.matmul(
                            out=psum[:, tt * K:(tt + 1) * K],
                            lhsT=x8s[c][:, bb * TPB + tt, :, :],
                            rhs=ctf8[0:66, :, :],
                            start=True, stop=True, perf_mode=DR,
                            skip_group_check=True)
                        pe_pin(i_mm)

                    i_sqrt = nc.scalar.activation(
                        out=dd[:, b * TPB:(b + 1) * TPB, :],
                        in_=psum[:].rearrange("p (t k) -> p t k", k=K),
                        func=mybir.ActivationFunctionType.Sqrt)
                    sqrts.append(i_sqrt)

                    ohm = ohp.tile([128, TPB, 2, K], f16, tag="ohm")
                    i_oh = nc.vector.tensor_tensor(
                        out=ohm[:, :, 0, :],
                        in0=iota_tbk,
                        in1=labf_sb[:, b * TPB:(b + 1) * TPB, None].to_broadcast(
                            (128, TPB, K)),
                        op=mybir.AluOpType.is_equal)
                    dve_pin(i_oh)
                    i_moh = nc.vector.tensor_tensor(
                        out=ohm[:, :, 1, :],
                        in0=dd[:, b * TPB:(b + 1) * TPB, :],
                        in1=ohm[:, :, 0, :],
                        op=mybir.AluOpType.mult)
                    dve_pin(i_moh)

                    blocks.append((b, ohm, i_moh))
                    if b >= 2:
                        emit_g(blocks[b - 2], False)

            emit_g(blocks[NB - 2], False)
            i_g_last = emit_g(blocks[NB - 1], True)

            # ---- finale ----
            pn2 = nc.gpsimd.nop()
            add_dep_helper(pn2.ins, casts[-1].ins, sync=True,
                           reason="pool observes casts")
            G_sb = sb.tile([K, 2 * K], f32)
            i_gc = nc.vector.tensor_copy(out=G_sb, in_=G_ps[:])
            dve_pin(i_gc)
            e1 = nc.gpsimd.dma_start(out=out_G[:], in_=G_sb)
            add_dep_helper(e1.ins, pn2.ins, sync=False, reason="pin")
            for inst in [i_g_last, sqrts[-1], i_gc, e1, casts[-1]]:
                n = nc.sync.nop()
                add_dep_helper(n.ins, inst.ins, sync=True, reason="end chain")
    _strip_redundant_waits(nc)
    return nc


def _two_digit(v):
    """v -> (16*a, fp8(r)) with v ~ 16*a + r; 16*a exact in e4m3."""
    a = np.clip(np.floor(v / 16.0 + 0.5), 0.0, 14.0).astype(np.float32)
    hi = 16.0 * a
    lo = (v - hi).astype(np.float32).astype(E4M3)
    return hi, lo


def _host_prep(centroids):
    c = np.ascontiguousarray(centroids, dtype=np.float32)          # [K, D]
    ct = (-2.0 * c).astype(E4M3)                                   # [K, D] fp8
    ct_f = ct.astype(np.float32)
    # device-effective centroids: c_eff[d, j] = -ct[j, d]/2 (exact /2)
    c_eff = -0.5 * ct_f.T                                          # [D, K]
    c2t = (c_eff.astype(np.float64) ** 2).sum(0).astype(np.float32)  # [K]
    c2hi, c2lo8 = _two_digit(c2t)
    c2_dev = c2hi + c2lo8.astype(np.float32)

    ctf8 = np.zeros((128, 2, K), dtype=E4M3)
    # ct packed: ctf8[p, q, j] = ct[j, q*64+p]
    ctf8[0:64] = ct.T.reshape(2, 64, K).transpose(1, 0, 2)
    ctf8[64, 0, :] = E4M3(1.0)
    ctf8[64, 1, :] = c2hi.astype(E4M3)          # 16*c2a, exact
    ctf8[65, 0, :] = c2lo8
    ctf8[65, 1, :] = E4M3(1.0)

    ctsum = ct_f.sum(0)                          # [D]: sum_j (-2 c_eff[., j])
    c2s = float(c2_dev.astype(np.float64).sum())
    return ctf8, ctsum, c2s


def _pack_consts(labf, ctf8):
    """Per-partition pack: iota[512 f16] labf[256 f16] ctf8[128 u8]."""
    iota = np.tile(np.arange(K, dtype=np.float16), TPB)            # [512]
    iota_b = np.broadcast_to(iota, (128, TPB * K))

    def u8(a):
        return np.ascontiguousarray(a).view(np.uint8)
    ct_rows = np.ascontiguousarray(ctf8.reshape(128, 2 * K)).view(np.uint8)
    rows = []
    for p in range(128):
        rows.append(np.concatenate([u8(iota_b[p]), u8(labf[p]), ct_rows[p]]))
    buf = np.stack(rows)                          # [128, 1664] u8
    return np.ascontiguousarray(buf).view(np.float32)


def kernel(embeddings, cluster_labels, centroids):
    embeddings = np.ascontiguousarray(embeddings, dtype=np.float32)
    cluster_labels = np.ascontiguousarray(cluster_labels, dtype=np.int64)
    centroids = np.ascontiguousarray(centroids, dtype=np.float32)

    if "nc" not in _cache:
        _cache["nc"] = _build()
    nc = _cache["nc"]

    ctf8, ctsum, c2s = _host_prep(centroids)

    x8_full = embeddings.astype(E4M3)                             # [N, D] fp8
    x2_full = (embeddings.astype(np.float64) ** 2).sum(1).astype(np.float32)
    x2hi, x2lo8 = _two_digit(x2_full)
    x2_dev = x2hi + x2lo8.astype(np.float32)                      # [N]

    in_maps = []
    for cix in range(NCORES):
        sl = slice(cix * NS, (cix + 1) * NS)
        x8 = x8_full[sl]                                          # [NS, D]
        ls = cluster_labels[sl]
        labf = np.ascontiguousarray(
            ls.reshape(T, 128).T).astype(np.float16)              # [128, T]

        arena = np.zeros((66, T, 2, 128), dtype=E4M3)
        # x packed: arena[p, t, q, m] = x8[t*128+m, q*64+p]
        arena[0:64] = x8.reshape(T, 128, 2, 64).transpose(3, 0, 2, 1)
        arena[64, :, 0, :] = x2hi[sl].astype(E4M3).reshape(T, 128)
        arena[64, :, 1, :] = E4M3(1.0)
        arena[65, :, 0, :] = E4M3(1.0)
        arena[65, :, 1, :] = x2lo8[sl].reshape(T, 128)

        consts = _pack_consts(labf, ctf8)
        in_maps.append({
            "x8": np.ascontiguousarray(arena.reshape(66, T * 2 * 128)),
            "consts": consts,
        })
    res = run_bass_kernel_spmd(nc, in_maps, core_ids=list(range(NCORES)))
    _cache["last_res"] = res

    G = np.zeros((K, 2 * K), np.float64)
    for r in res.results:
        G += r["out_G"].astype(np.float64)
    G1 = G[:, 0:K]
    G2 = G[:, K:2 * K]

    labels = cluster_labels
    counts = np.bincount(labels, minlength=K).astype(np.float64)
    O = np.diag(G1)                               # sum_{n in k} dist_own
    S1 = G1.sum(0)                                # sum_{n in k} sum_j dist
    W = S1 - O
    A = np.diag(G2)                               # sum_{n in k} d2_own

    # closed-form S2_k = sum_{n in k} sum_j d2[n, j], device-consistent
    q = x8_full.astype(np.float32) @ ctsum        # [N]: -2 x.(sum_j c_j)
    sx2 = np.bincount(labels, weights=x2_dev.astype(np.float64), minlength=K)
    sq = np.bincount(labels, weights=q.astype(np.float64), minlength=K)
    S2 = K * sx2 + counts * c2s + sq
    V = S2 - A

    safe = np.maximum(counts, 1.0)
    t_k = A + (MARGIN * MARGIN * (K - 1) * counts - 2.0 * MARGIN * W + V) / (K - 1)
    loss = np.where(counts > 0, t_k / safe, 0.0).sum() / K
    return np.float32(loss)
